# revision 1
# baseline (speedup 1.0000x reference)
"""Trainium2 Bass kernel for nn_AdjCompute (pairwise |x_i-x_j| -> 4x(1x1 conv+BN+lrelu) -> 1x1 conv).

v2: wrapped-band symmetric version. out[i,j] == out[j,i], so each 8-row group
g computes only a cyclic column window [8g, 8g + W_g) mod 1536 with
W_g = 776 for g < 96 and 768 for g >= 96. This covers every off-diagonal
8x8 block pair exactly once and every diagonal block fully; the mirror half
is assembled on the host. BN statistics use
  S_full = 2*S_computed - S_diagblocks.
All cores get identical op shapes (12 groups of each width class); per-core
variation (which rows, which wrapped columns) is carried entirely by input
data (xw = pre-gathered wrapped x columns, xp = pair scalars).

Device layout (per core, 24 groups, total computed cols WTA = 18528):
  stage A (64->16->16 ch): flat column stream; group gi at stream cols
    [OFF[gi], OFF[gi]+W), partition = 16*r + o (row-in-group, channel).
  stage B (16->8->8->1 ch): stream halves stacked: partition = 64*u + 8*r + o,
    stage-B col c in [0, 9264): u=0 <-> stage-A col c, u=1 <-> 9264 + c.
Output: raw [128, 2688] f32 stage-B stream dump per core; host unscrambles
and mirrors.
"""

import numpy as np

from concourse import bacc, mybir, tile
from concourse.bass_utils import run_bass_kernel_spmd

NC_ = 8
N = 1536
NTOT = float(N * N)
EPS = 1e-5
SLOPE = 0.01
GPC = 24  # groups per core

f32, f16 = mybir.dt.float32, mybir.dt.float16
A = mybir.AluOpType
AF = mybir.ActivationFunctionType

_CACHE = {}
LAST_EXEC_NS = None


def _glist(core):
    gl = []
    for t in range(12):
        gl.append(core + 8 * t)  # W = 776
        gl.append(96 + core + 8 * t)  # W = 768
    return gl


_LL = [776 if i % 2 == 0 else 768 for i in range(GPC)]  # identical for all cores
_OFF = np.concatenate([[0], np.cumsum(_LL)]).astype(int)
WTA = int(_OFF[-1])  # 18528
WTB = WTA // 2  # 9264
assert int(_OFF[12]) == WTB

# stage-A per-group tiling (chunks of <=512, one PSUM bank) for mm1/copy/stats
TILE_A = []  # (gi, stream_start, width)
for gi in range(GPC):
    L = _LL[gi]
    c = 0
    while c < L:
        w = min(512, L - c)
        TILE_A.append((gi, int(_OFF[gi]) + c, w))
        c += w
NTA = len(TILE_A)  # 48

# flat stage-A tiling for mm2 (512 chunks)
TILE_F = []
c = 0
while c < WTA:
    w = min(512, WTA - c)
    TILE_F.append((c, w))
    c += w
NTF = len(TILE_F)  # 37

SLAB_A = []
c = 0
while c < WTA:
    w = min(1536, WTA - c)
    SLAB_A.append((c, w))
    c += w

# stage-B tiling (384 chunks)
TILE_B = []
c = 0
while c < WTB:
    w = min(384, WTB - c)
    TILE_B.append((c, w))
    c += w
NTB = len(TILE_B)  # 25
NP5 = (NTB + 3) // 4  # 7 psum5 tiles
WOUT = NP5 * 384  # 2688

SLAB_B = []
c = 0
while c < WTB:
    w = min(1536, WTB - c)
    SLAB_B.append((c, w))
    c += w


def _build():
    nc = bacc.Bacc("TRN2", target_bir_lowering=False, debug=False, num_devices=NC_)

    def din(name, shape, dt):
        return nc.dram_tensor(name, shape, dt, kind="ExternalInput")

    xe_e = din("xe", [128, 2240], f16)
    xp_e = din("xp", [128, 96], f32)
    l1_e = din("lhsT1", [128, 32], f16)
    l1n_e = din("lhsT1n", [128, 32], f16)
    l2_e = din("lhsT2", [128, 128], f16)
    l3_e = din("lhsT3", [128, 64], f16)
    l4_e = din("lhsT4", [128, 128], f16)
    l5_e = din("lhsT5", [128, 16], f16)
    p16_e = din("pat16", [128, 128], f32)
    p8_e = din("pat8", [128, 128], f32)
    gb_e = din("gb", [128, 8], f32)
    b5_e = din("b5b", [128, 1], f32)
    out_e = nc.dram_tensor("out", [128, WOUT], f32, kind="ExternalOutput")

    with tile.TileContext(nc) as tc:
        with (
            tc.tile_pool(name="const", bufs=1) as cpool,
            tc.tile_pool(name="big", bufs=3) as big,
            tc.tile_pool(name="adjp", bufs=4) as adjp,
            tc.tile_pool(name="dtp", bufs=2) as dtp,
            tc.tile_pool(name="atp", bufs=3) as atp,
            tc.tile_pool(name="jkp", bufs=2) as jkp,
            tc.tile_pool(name="statp", bufs=1) as statp,
            tc.tile_pool(name="smallp", bufs=1) as smallp,
            tc.tile_pool(name="outp", bufs=1) as outp,
            tc.tile_pool(name="psA", bufs=7, space="PSUM") as psA,
            tc.tile_pool(name="psS", bufs=1, space="PSUM") as psS,
            tc.tile_pool(name="dram", bufs=1, space="DRAM") as dram,
        ):
            # ---- constants ----
            xp = cpool.tile([128, 96], f32)
            l1 = cpool.tile([128, 32], f16)
            l1n = cpool.tile([128, 32], f16)
            l2 = cpool.tile([128, 128], f16)
            l3 = cpool.tile([128, 64], f16)
            l4 = cpool.tile([128, 128], f16)
            l5 = cpool.tile([128, 16], f16)
            p16 = cpool.tile([128, 128], f32)
            p8 = cpool.tile([128, 128], f32)
            gb = cpool.tile([128, 8], f32)
            b5b = cpool.tile([128, 1], f32)
            for t, e in [
                (xp, xp_e), (l1, l1_e), (l1n, l1n_e), (l2, l2_e),
                (l3, l3_e), (l4, l4_e), (l5, l5_e), (p16, p16_e), (p8, p8_e),
                (gb, gb_e), (b5b, b5_e),
            ]:
                sl = (slice(None),) * len(t.shape)
                nc.sync.dma_start(t[sl], e[sl])

            # warmup collective: absorbs the cold-start cost of the CC path
            wrm = smallp.tile([128, 2], f32, name="wrm")
            nc.vector.memset(wrm[:, :], 0.0)
            agiw = dram.tile([128, 2], f32, name="agiw")
            agow = dram.tile([128 * NC_, 2], f32, addr_space="Shared", name="agow")
            nc.sync.dma_start(agiw[:, :], wrm[:, :])
            nc.gpsimd.collective_compute(
                "AllGather", A.bypass,
                replica_groups=[list(range(NC_))],
                ins=[agiw.opt()], outs=[agow.opt()],
            )

            # rotated x columns: xe[:, j] = xT[:, (8*core + j) % N] (host-built),
            # so group gi's window is the build-time slice [rot(gi), rot(gi)+W)
            xe = cpool.tile([128, 2240], f16)
            nc.sync.dma_start(xe[:, :], xe_e[:, :])

            h1 = big.tile([128, WTA], f16, tag="hbuf")

            sumb = {}
            sqb = {}
            dsb = {}
            dqb = {}
            stbn = {}
            n_bn = {}
            n_s2 = {}
            for k, nt in [(1, NTA), (2, NTF), (3, NTB), (4, NTB)]:
                sumb[k] = statp.tile([128, nt], f32, name=f"sumb{k}")
                sqb[k] = statp.tile([128, nt], f32, name=f"sqb{k}")
                stbn[k] = statp.tile([128, 6 * nt], f32, name=f"stbn{k}")
                dsb[k] = statp.tile([128, 4], f32, name=f"dsb{k}")
                dqb[k] = statp.tile([128, 4], f32, name=f"dqb{k}")
                nc.vector.memset(dsb[k][:, :], 0.0)
                nc.vector.memset(dqb[k][:, :], 0.0)
                n_bn[k] = 0
                n_s2[k] = 0
            w_bn = {1: 0, 2: 0, 3: 0, 4: 0}

            def copy_and_stats(k, ti, ps, wid, dst, eng):
                if k >= 2 and ti % 3 == 2:
                    nc.vector.tensor_scalar(
                        out=dst, in0=ps, scalar1=1.0, scalar2=None, op0=A.mult,
                    )
                else:
                    nc.scalar.activation(out=dst, in_=ps, func=AF.Copy)
                j = n_bn[k]
                n_bn[k] += 1
                w_bn[k] += wid
                nc.vector.bn_stats(stbn[k][:, 6 * j : 6 * j + 6], dst)

            def diag_stats_batched(k, hst, stage):
                # diag blocks of group gi start at stream col OFF[gi]:
                # {1544*t, 1544*t + 776} = 8*(193*t + {0, 97}).
                # stage A: 12 t-blocks over full 128 partitions;
                # stage B: 6 t-blocks per u-half (u=0: partitions 0:64, u=1: 64:128).
                if stage == 0:
                    nt = 12
                    view = hst.rearrange("p (t q j) -> p t q j", t=nt, q=193, j=8)
                    parts = [(0, 128)]
                else:
                    nt = 6
                    view = hst.rearrange("p (t q j) -> p t q j", t=nt, q=193, j=8)
                    parts = [(0, 64), (64, 64)]
                col = -1
                for p0, pn in parts:
                    for qi in (0, 97):
                        col += 1
                        jd = smallp.tile(
                            [128, 12, 8], f16, name=f"jd{k}_{col}_{p0}", tag="jd"
                        )
                        nc.vector.tensor_scalar(
                            out=jd[p0 : p0 + pn, :nt, :],
                            in0=view[p0 : p0 + pn, :, qi, :],
                            scalar1=0.5, scalar2=0.0, op0=A.mult, op1=A.add,
                            accum_out=dsb[k][p0 : p0 + pn, col : col + 1],
                        )
                        jd2 = smallp.tile(
                            [128, 12, 8], f16, name=f"jd2{k}_{col}_{p0}", tag="jd2"
                        )
                        nc.vector.scalar_tensor_tensor(
                            out=jd2[p0 : p0 + pn, :nt, :],
                            in0=view[p0 : p0 + pn, :, qi, :],
                            scalar=0.5, in1=view[p0 : p0 + pn, :, qi, :],
                            op0=A.mult, op1=A.mult,
                            accum_out=dqb[k][p0 : p0 + pn, col : col + 1],
                        )


            # ---- barrier (split: early AG over the first n1 tiles, hidden
            #      under the pass tail; final AG over the rest + diag) ----
            def barrier_stage1(k, pat):
                # called mid-pass once n_bn[k] tiles have stats; AG them early
                n1 = n_bn[k]
                w1 = w_bn[k]
                ba = smallp.tile([128, 2], f32, name=f"ba_a{k}")
                nc.vector.bn_aggr(ba[:, :], stbn[k][:, : 6 * n1])
                m2 = smallp.tile([128, 1], f32, name=f"m2a_{k}")
                nc.vector.tensor_tensor(
                    out=m2[:, :], in0=ba[:, 0:1], in1=ba[:, 0:1], op=A.mult,
                )
                sq = smallp.tile([128, 2], f32, name=f"sqa{k}")
                # col0 = -S1 ; col1 = Q1 = w1*(v + m^2)
                nc.vector.tensor_scalar(
                    out=sq[:, 0:1], in0=ba[:, 0:1], scalar1=float(-w1),
                    scalar2=None, op0=A.mult,
                )
                q1 = smallp.tile([128, 1], f32, name=f"q1a_{k}")
                nc.vector.tensor_tensor(
                    out=q1[:, :], in0=ba[:, 1:2], in1=m2[:, :], op=A.add,
                )
                nc.vector.tensor_scalar(
                    out=sq[:, 1:2], in0=q1[:, :], scalar1=float(w1),
                    scalar2=None, op0=A.mult,
                )
                pf = psS.tile([128, 2], f32, tag="psS", name=f"pfa{k}")
                nc.tensor.matmul(pf[:, :], pat[:, :], sq[:, :], start=True, stop=True)
                gl = smallp.tile([128, 2], f32, name=f"gla{k}")
                nc.vector.tensor_copy(gl[:, :], pf[:, :])
                agi = dram.tile([128, 2], f32, name=f"agia{k}")
                ago = dram.tile([128 * NC_, 2], f32, addr_space="Shared", name=f"agoa{k}")
                nc.sync.dma_start(agi[:, :], gl[:, :])
                nc.gpsimd.collective_compute(
                    "AllGather", A.bypass,
                    replica_groups=[list(range(NC_))],
                    ins=[agi.opt()], outs=[ago.opt()],
                )
                agva = smallp.tile([128, 2, NC_], f32, name=f"agva{k}")
                nc.sync.dma_start(
                    agva[:, :, :], ago.rearrange("(b p) c -> p c b", b=NC_),
                )
                gta = smallp.tile([128, 2], f32, name=f"gta{k}")
                nc.vector.tensor_reduce(
                    out=gta[:, :], in_=agva[:, :, :],
                    axis=mybir.AxisListType.X, op=A.add,
                )
                return n1, w1, gta

            def barrier(k, pat, gcol, becol, stage1):
                if stage1 is None:
                    n1, w1, gta = 0, 0, None
                else:
                    n1, w1, gta = stage1
                n2, w2 = n_bn[k] - n1, w_bn[k] - w1
                sq = smallp.tile([128, 2], f32, name=f"sq{k}")
                ba = smallp.tile([128, 2], f32, name=f"ba{k}")
                nc.vector.bn_aggr(ba[:, :], stbn[k][:, 6 * n1 : 6 * n_bn[k]])
                m2 = smallp.tile([128, 1], f32, name=f"m2_{k}")
                nc.vector.tensor_tensor(
                    out=m2[:, :], in0=ba[:, 0:1], in1=ba[:, 0:1], op=A.mult,
                )
                q1 = smallp.tile([128, 1], f32, name=f"q1_{k}")
                nc.vector.tensor_tensor(
                    out=q1[:, :], in0=ba[:, 1:2], in1=m2[:, :], op=A.add,
                )
                ds_l = smallp.tile([128, 1], f32, name=f"dsl{k}")
                nc.vector.tensor_reduce(
                    out=ds_l[:, :], in_=dsb[k][:, :],
                    axis=mybir.AxisListType.X, op=A.add,
                )
                dq_l = smallp.tile([128, 1], f32, name=f"dql{k}")
                nc.vector.tensor_reduce(
                    out=dq_l[:, :], in_=dqb[k][:, :],
                    axis=mybir.AxisListType.X, op=A.add,
                )
                # col0 = DS/2 - S2 ; col1 = Q2 - DQ/2
                nc.vector.scalar_tensor_tensor(
                    out=sq[:, 0:1], in0=ba[:, 0:1], scalar=float(-w2), in1=ds_l[:, :],
                    op0=A.mult, op1=A.add,
                )
                nc.vector.scalar_tensor_tensor(
                    out=sq[:, 1:2], in0=q1[:, :], scalar=float(w2), in1=dq_l[:, :],
                    op0=A.mult, op1=A.subtract,
                )
                pf = psS.tile([128, 2], f32, tag="psS", name=f"pf{k}")
                nc.tensor.matmul(pf[:, :], pat[:, :], sq[:, :], start=True, stop=True)
                gl = smallp.tile([128, 2], f32, name=f"gl{k}")
                nc.vector.tensor_copy(gl[:, :], pf[:, :])
                agi = dram.tile([128, 2], f32, name=f"agi{k}")
                ago = dram.tile([128 * NC_, 2], f32, addr_space="Shared", name=f"ago{k}")
                nc.sync.dma_start(agi[:, :], gl[:, :])
                nc.gpsimd.collective_compute(
                    "AllGather", A.bypass,
                    replica_groups=[list(range(NC_))],
                    ins=[agi.opt()], outs=[ago.opt()],
                )
                agv = smallp.tile([128, 2, NC_], f32, name=f"agv{k}")
                nc.sync.dma_start(
                    agv[:, :, :], ago.rearrange("(b p) c -> p c b", b=NC_),
                )
                if gta is None:
                    gt = smallp.tile([128, 2], f32, name=f"gt{k}")
                    nc.vector.tensor_reduce(
                        out=gt[:, :], in_=agv[:, :, :],
                        axis=mybir.AxisListType.X, op=A.add,
                    )
                else:
                    gtb = smallp.tile([128, 2], f32, name=f"gtb{k}")
                    nc.vector.tensor_reduce(
                        out=gtb[:, :], in_=agv[:, :, :],
                        axis=mybir.AxisListType.X, op=A.add,
                    )
                    gt = smallp.tile([128, 2], f32, name=f"gt{k}")
                    nc.vector.tensor_tensor(
                        out=gt[:, :], in0=gta[:, :], in1=gtb[:, :], op=A.add,
                    )
                # pats are pre-scaled by 2/NTOT: gt0 = -mean, gt1 = E[h^2]
                negmean = gt[:, 0:1]
                msq = smallp.tile([128, 1], f32, name=f"ms{k}")
                nc.vector.tensor_tensor(
                    out=msq[:, :], in0=gt[:, 0:1], in1=gt[:, 0:1], op=A.mult,
                )
                ex2e = smallp.tile([128, 1], f32, name=f"ex{k}")
                nc.vector.tensor_scalar(
                    out=ex2e[:, :], in0=gt[:, 1:2], scalar1=EPS,
                    scalar2=None, op0=A.add,
                )
                vpe = smallp.tile([128, 1], f32, name=f"vp{k}")
                nc.vector.scalar_tensor_tensor(
                    out=vpe[:, :], in0=msq[:, :], scalar=-1.0, in1=ex2e[:, :],
                    op0=A.mult, op1=A.add,
                )
                rinv = smallp.tile([128, 1], f32, name=f"ri{k}")
                nc.vector.reciprocal(rinv[:, :], vpe[:, :])
                rstd = smallp.tile([128, 1], f32, name=f"rs{k}")
                nc.scalar.activation(out=rstd[:, :], in_=rinv[:, :], func=AF.Sqrt)
                sk = smallp.tile([128, 1], f32, name=f"s{k}")
                nc.vector.tensor_tensor(
                    out=sk[:, :], in0=rstd[:, :], in1=gb[:, gcol : gcol + 1], op=A.mult,
                )
                tk = smallp.tile([128, 1], f32, name=f"t{k}")
                nc.vector.scalar_tensor_tensor(
                    out=tk[:, :], in0=sk[:, :], scalar=negmean,
                    in1=gb[:, becol : becol + 1], op0=A.mult, op1=A.add,
                )
                return sk, tk

            # ================= PASS 1 =================
            gi2tiles = {}
            for ti, (gi, c0, w) in enumerate(TILE_A):
                gi2tiles.setdefault(gi, []).append((ti, c0, w))

            for gi in range(GPC):
                L = _LL[gi]
                o0 = int(_OFF[gi])
                rot = 64 * (gi // 2) + (768 if gi % 2 == 1 else 0)
                adjs = []
                for pp in range(4):
                    idx = 4 * gi + pp
                    adj = adjp.tile([128, 776], f16, tag="adj", name=f"adj_{idx}")
                    if idx % 2 == 0:
                        nc.scalar.activation(
                            out=adj[:, :L], in_=xe[:, rot : rot + L], func=AF.Abs,
                            bias=xp[:, idx : idx + 1], scale=-1.0,
                        )
                    else:
                        d = dtp.tile([128, 776], f16, tag="dt", name=f"d_{idx}")
                        nc.vector.tensor_scalar(
                            out=d[:, :L], in0=xe[:, rot : rot + L],
                            scalar1=xp[:, idx : idx + 1], scalar2=None,
                            op0=A.subtract,
                        )
                        nc.vector.scalar_tensor_tensor(
                            out=adj[:, :L], in0=d[:, :L], scalar=-1.0, in1=d[:, :L],
                            op0=A.mult, op1=A.max,
                        )
                    adjs.append(adj)
                for ti, c0, w in gi2tiles[gi]:
                    lc = c0 - o0
                    ps = psA.tile([128, 512], f32, tag="psA", name=f"h1p_{ti}")
                    for pp in range(4):
                        nc.tensor.matmul(
                            ps[32 * pp : 32 * pp + 32, :w],
                            l1[:, :],
                            adjs[pp][:, lc : lc + w],
                            start=True, stop=True,
                            tile_position=(0, 32 * pp),
                        )
                    copy_and_stats(1, ti, ps[:, :w], w, h1[:, c0 : c0 + w], False)
            with tc.high_priority():
                diag_stats_batched(1, h1, 0)

            with tc.high_priority():
                s1, t1 = barrier(1, p16, 0, 1, None)

            # ================= PASS 2: apply1, mm2, h2 =================
            h2 = big.tile([128, WTA], f16, tag="hbuf")
            for si, (c0, w) in enumerate(SLAB_A):
                at = atp.tile([128, 1536], f16, tag="at", name=f"a1_{si}")
                if si % 3 == 2:
                    u1t = dtp.tile([128, 1536], f16, tag="dt2", name=f"u1_{si}")
                    nc.vector.tensor_scalar(
                        out=u1t[:, :w], in0=h1[:, c0 : c0 + w], scalar1=s1[:, :],
                        scalar2=t1[:, :], op0=A.mult, op1=A.add,
                    )
                    nc.vector.scalar_tensor_tensor(
                        out=at[:, :w], in0=u1t[:, :w], scalar=SLOPE,
                        in1=u1t[:, :w], op0=A.mult, op1=A.max,
                    )
                else:
                    nc.scalar.activation(
                        out=at[:, :w], in_=h1[:, c0 : c0 + w],
                        func=AF.Lrelu, scale=s1[:, :], bias=t1[:, :], alpha=SLOPE,
                    )
                for z in range(0, w, 512):
                    wz = min(512, w - z)
                    ti = (c0 + z) // 512
                    ps = psA.tile([128, 512], f32, tag="psA", name=f"h2p_{ti}")
                    nc.tensor.matmul(
                        ps[:, :wz], l2[:, :], at[:, z : z + wz],
                        start=True, stop=True,
                    )
                    copy_and_stats(
                        2, ti, ps[:, :wz], wz, h2[:, c0 + z : c0 + z + wz], False
                    )
            with tc.high_priority():
                diag_stats_batched(2, h2, 0)

            with tc.high_priority():
                s2, t2 = barrier(2, p16, 2, 3, None)

            # ================= PASS 3: apply2, mm3, h3 =================
            a2 = big.tile([128, WTA], f16, tag="hbuf")
            _order = [0, 6, 1, 7, 2, 8, 3, 9, 4, 10, 5, 11, 12]
            for si in _order:
                c0, w = SLAB_A[si]
                if si % 2 == 0:
                    nc.scalar.activation(
                        out=a2[:, c0 : c0 + w], in_=h2[:, c0 : c0 + w],
                        func=AF.Lrelu, scale=s2[:, :], bias=t2[:, :], alpha=SLOPE,
                    )
                else:
                    u = dtp.tile([128, 1536], f16, tag="dt2", name=f"u2_{si}")
                    nc.vector.tensor_scalar(
                        out=u[:, :w], in0=h2[:, c0 : c0 + w], scalar1=s2[:, :],
                        scalar2=t2[:, :], op0=A.mult, op1=A.add,
                    )
                    nc.vector.scalar_tensor_tensor(
                        out=a2[:, c0 : c0 + w], in0=u[:, :w], scalar=SLOPE,
                        in1=u[:, :w], op0=A.mult, op1=A.max,
                    )
            h3 = big.tile([128, WTB], f16, tag="hbuf")
            for ti, (c0, w) in enumerate(TILE_B):
                ps = psA.tile([128, 384], f32, tag="psA", name=f"h3p_{ti}")
                for u in range(2):
                    nc.tensor.matmul(
                        ps[64 * u : 64 * u + 64, :w],
                        l3[:, :],
                        a2[:, WTB * u + c0 : WTB * u + c0 + w],
                        start=True, stop=True,
                        tile_position=(0, 64 * u),
                    )
                copy_and_stats(3, ti, ps[:, :w], w, h3[:, c0 : c0 + w], False)
            with tc.high_priority():
                diag_stats_batched(3, h3, 1)

            with tc.high_priority():
                s3, t3v = barrier(3, p8, 4, 5, None)

            # ================= PASS 4: apply3, mm4, h4 =================
            h4 = big.tile([128, WTB], f16, tag="hbuf")
            for si, (c0, w) in enumerate(SLAB_B):
                at = atp.tile([128, 1536], f16, tag="at", name=f"a3_{si}")
                nc.scalar.activation(
                    out=at[:, :w], in_=h3[:, c0 : c0 + w],
                    func=AF.Lrelu, scale=s3[:, :], bias=t3v[:, :], alpha=SLOPE,
                )
                for z in range(0, w, 384):
                    wz = min(384, w - z)
                    ti = (c0 + z) // 384
                    ps = psA.tile([128, 384], f32, tag="psA", name=f"h4p_{ti}")
                    nc.tensor.matmul(
                        ps[:, :wz], l4[:, :], at[:, z : z + wz],
                        start=True, stop=True,
                    )
                    copy_and_stats(
                        4, ti, ps[:, :wz], wz, h4[:, c0 + z : c0 + z + wz], False
                    )
            with tc.high_priority():
                diag_stats_batched(4, h4, 1)

            with tc.high_priority():
                s4, t4v = barrier(4, p8, 6, 7, None)

            # ================= PASS 5: apply4, mm5, out =================
            outb = outp.tile([128, WOUT], f32)
            for pi in range(NP5):
                ps5 = psA.tile([128, 384], f32, tag="psA", name=f"h5p_{pi}")
                for k in range(4):
                    ti = 4 * pi + k
                    if ti >= NTB:
                        nc.vector.memset(ps5[32 * k : 32 * k + 16, :], 0.0)
                        continue
                    c0, w = TILE_B[ti]
                    at = atp.tile([128, 1536], f16, tag="at", name=f"a4_{ti}")
                    if ti % 3 == 2:
                        u = dtp.tile([128, 1536], f16, tag="dt2", name=f"u4_{ti}")
                        nc.vector.tensor_scalar(
                            out=u[:, :w], in0=h4[:, c0 : c0 + w], scalar1=s4[:, :],
                            scalar2=t4v[:, :], op0=A.mult, op1=A.add,
                        )
                        nc.vector.scalar_tensor_tensor(
                            out=at[:, :w], in0=u[:, :w], scalar=SLOPE,
                            in1=u[:, :w], op0=A.mult, op1=A.max,
                        )
                    else:
                        nc.scalar.activation(
                            out=at[:, :w], in_=h4[:, c0 : c0 + w],
                            func=AF.Lrelu, scale=s4[:, :], bias=t4v[:, :], alpha=SLOPE,
                        )
                    nc.tensor.matmul(
                        ps5[32 * k : 32 * k + 16, :w], l5[:, :], at[:, :w],
                        start=True, stop=True,
                        tile_position=(0, 32 * k),
                    )
                    if w < 384:
                        nc.vector.memset(ps5[32 * k : 32 * k + 16, w:384], 0.0)
                nc.scalar.activation(
                    out=outb[:, 384 * pi : 384 * pi + 384], in_=ps5[:, :],
                    func=AF.Identity, bias=b5b[:, :], scale=1.0,
                )
                nc.sync.dma_start(
                    out_e[:, 384 * pi : 384 * pi + 384],
                    outb[:, 384 * pi : 384 * pi + 384],
                )

    nc.compile()
    return nc


def _host_inputs(x, W1, W2, W3, W4, W5, g1, be1, g2, be2, g3, be3, g4, be4, b5):
    xT = x.T.astype(np.float32)  # [64, 1536]

    lhsT1 = np.zeros((128, 32), np.float32)
    for d in range(2):
        lhsT1[64 * d : 64 * d + 64, 16 * d : 16 * d + 16] = W1.T
    lhsT2 = np.zeros((128, 128), np.float32)
    for r in range(8):
        lhsT2[16 * r : 16 * r + 16, 16 * r : 16 * r + 16] = W2.T
    lhsT3 = np.zeros((128, 64), np.float32)
    for r in range(8):
        lhsT3[16 * r : 16 * r + 16, 8 * r : 8 * r + 8] = W3.T
    lhsT4 = np.zeros((128, 128), np.float32)
    for b in range(16):
        lhsT4[8 * b : 8 * b + 8, 8 * b : 8 * b + 8] = W4.T
    lhsT5 = np.zeros((128, 16), np.float32)
    for b in range(16):
        lhsT5[8 * b : 8 * b + 8, b] = W5[0, :]

    q = np.arange(128)
    pat16 = (q[:, None] % 16 == q[None, :] % 16).astype(np.float32) * (2.0 / NTOT)
    pat8 = (q[:, None] % 8 == q[None, :] % 8).astype(np.float32) * (2.0 / NTOT)
    gb = np.stack(
        [
            g1[q % 16], be1[q % 16], g2[q % 16], be2[q % 16],
            g3[q % 8], be3[q % 8], g4[q % 8], be4[q % 8],
        ],
        axis=1,
    ).astype(np.float32)
    b5b = np.full((128, 1), float(b5[0]), np.float32)

    common = {
        "lhsT1": lhsT1.astype(np.float16),
        "lhsT1n": (-lhsT1).astype(np.float16),
        "lhsT2": lhsT2.astype(np.float16),
        "lhsT3": lhsT3.astype(np.float16),
        "lhsT4": lhsT4.astype(np.float16),
        "lhsT5": lhsT5.astype(np.float16),
        "pat16": pat16,
        "pat8": pat8,
        "gb": gb,
        "b5b": b5b,
    }

    in_maps = []
    for core in range(NC_):
        gl = _glist(core)
        cols = (8 * core + np.arange(2240)) % N
        xe = xT[:, cols]
        xp = np.zeros((128, 96), np.float32)
        for gi, g in enumerate(gl):
            for pp in range(4):
                for d in range(2):
                    xp[64 * d : 64 * d + 64, 4 * gi + pp] = x[8 * g + 2 * pp + d, :]
        m = dict(common)
        m["xe"] = np.concatenate([xe, xe], axis=0).astype(np.float16)
        m["xp"] = xp
        in_maps.append(m)
    return in_maps


def _decode_maps():
    """Static scatter maps: (core, partition, outcol) -> (row, col) of out[N,N]."""
    if "maps" in _CACHE:
        return _CACHE["maps"]
    rows = np.zeros((NC_, 128, WOUT), np.int32)
    cols = np.zeros((NC_, 128, WOUT), np.int32)
    valid = np.zeros((NC_, 128, WOUT), bool)
    for core in range(NC_):
        gl = _glist(core)
        for ti, (cb, w) in enumerate(TILE_B):
            pi, k = ti // 4, ti % 4
            for u in range(2):
                cA0 = WTB * u + cb
                for gi in range(GPC):
                    lo = max(int(_OFF[gi]), cA0)
                    hi = min(int(_OFF[gi + 1]), cA0 + w)
                    if lo >= hi:
                        continue
                    g = gl[gi]
                    jj = np.arange(lo, hi)
                    j = (8 * g + (jj - int(_OFF[gi]))) % N
                    oc = 384 * pi + (jj - cA0)
                    for r in range(8):
                        p = 32 * k + 8 * u + r
                        rows[core, p, oc] = 8 * g + r
                        cols[core, p, oc] = j
                        valid[core, p, oc] = True
    _CACHE["maps"] = (rows, cols, valid)
    return _CACHE["maps"]


def kernel(**inputs):
    global LAST_EXEC_NS
    import os

    x = np.asarray(inputs["x"], np.float32)
    args = [
        np.asarray(inputs[k], np.float32)
        for k in ("W1", "W2", "W3", "W4", "W5", "g1", "be1", "g2", "be2",
                  "g3", "be3", "g4", "be4", "b5")
    ]
    in_maps = _host_inputs(x, *args)

    if "nc" not in _CACHE:
        _CACHE["nc"] = _build()
    nc = _CACHE["nc"]

    trace = os.environ.get("KERNEL_TRACE", "0") == "1"
    res = run_bass_kernel_spmd(nc, in_maps, core_ids=list(range(NC_)), trace=trace)
    LAST_EXEC_NS = res.exec_time_ns

    rows, cols, valid = _decode_maps()
    out = np.zeros((N, N), np.float32)
    for core in range(NC_):
        raw = np.asarray(res.results[core]["out"])
        v = valid[core]
        out[rows[core][v], cols[core][v]] = raw[v]
    # mirror the uncovered orientations (covered set: every unordered pair once)
    if "mirror" not in _CACHE:
        cov = np.zeros((N, N), bool)
        for core in range(NC_):
            v = valid[core]
            cov[rows[core][v], cols[core][v]] = True
        _CACHE["mirror"] = ~cov
    m = _CACHE["mirror"]
    out[m] = out.T[m]
    return out

